# revision 18
# baseline (speedup 1.0000x reference)
"""Trainium2 Bass kernel for nn_AttentionBlock (GroupNorm + 4-head self-attention + proj).

Sharding: 8 cores; core i handles batch b=i//2 and pixel-half i%2 (2048 of 4096
pixels). Each core computes GroupNorm + k,v over the full batch image (needed
for attention over all keys), q/attention/proj only for its pixel half.
Host-side work is limited to slicing, transposing weight layouts, and
concatenating the 8 output shards.
"""

import sys

sys.path.insert(0, "/opt/trn_rl_repo")

import numpy as np

import concourse.bass as bass
import concourse.mybir as mybir
import concourse.tile as tile
from concourse import bacc
from concourse.bass_utils import run_bass_kernel_spmd

F32 = mybir.dt.float32
BF16 = mybir.dt.bfloat16
AF = mybir.ActivationFunctionType

B, C, H, W = 4, 256, 64, 64
N = H * W          # 4096 pixels
NHALF = N // 2     # 2048 per core
G = 8              # groupnorm groups
NHEADS = 4
HD = C // NHEADS   # 64
CT = C // 128      # 2 channel tiles of 128
SCALE = HD ** -0.5
EPS = 1e-5
CNT = (C // G) * N  # elements per group (per batch)


def build_nc(reps=1):
    nc = bacc.Bacc(None, target_bir_lowering=False)

    x_in = nc.declare_dram_parameter("xb", [C, N], F32, isOutput=False)
    wqkvT_in = nc.declare_dram_parameter("wqkvT", [C, 3 * C], F32, isOutput=False)
    wprojTh_in = nc.declare_dram_parameter("wprojTh", [NHEADS, HD, C], F32, isOutput=False)
    qkvb_in = nc.declare_dram_parameter("qkvb", [3 * C], F32, isOutput=False)
    vb_in = nc.declare_dram_parameter("vb", [NHEADS, HD], F32, isOutput=False)
    projb_in = nc.declare_dram_parameter("projb", [C], F32, isOutput=False)
    gamma_in = nc.declare_dram_parameter("gamma", [C], F32, isOutput=False)
    beta_in = nc.declare_dram_parameter("beta", [C], F32, isOutput=False)
    m8_in = nc.declare_dram_parameter("m8", [CT, 128, G], F32, isOutput=False)
    ind8_in = nc.declare_dram_parameter("ind8", [CT, G, 128], F32, isOutput=False)
    y_out = nc.declare_dram_parameter("y", [C, NHALF], F32, isOutput=True)

    x_t = x_in[:].rearrange("(t p) n -> t p n", p=128)
    w_t = wqkvT_in[:].rearrange("(t p) o -> t p o", p=128)
    y_t = y_out[:].rearrange("(t p) n -> t p n", p=128)

    with tile.TileContext(nc) as tc:
        with (
            tc.tile_pool(name="persist", bufs=1) as P1,
            tc.tile_pool(name="scratch", bufs=2) as SCR,
        ):
            import contextlib
            loop_cm = tc.For_i(0, reps, 1) if reps > 1 else contextlib.nullcontext()
            with loop_cm:
                # ---------- load ----------
                x_sb = [P1.tile([128, N], F32, tag=f"x{t}", name=f"x{t}") for t in range(CT)]
                NCH = 4  # chunked load so groupnorm stats can start early
                for t in range(CT):
                    for jc in range(NCH):
                        nc.sync.dma_start(
                            out=x_sb[t][:, jc * (N // NCH):(jc + 1) * (N // NCH)],
                            in_=x_t[t][:, jc * (N // NCH):(jc + 1) * (N // NCH)],
                        )

                wq_b = [P1.tile([128, 3 * C], BF16, tag=f"wq{t}", name=f"wq{t}") for t in range(CT)]
                for t in range(CT):
                    ws = SCR.tile([128, 3 * C], F32, tag="wstage", name="wstage")
                    nc.sync.dma_start(out=ws[:], in_=w_t[t])
                    nc.vector.tensor_copy(out=wq_b[t][:], in_=ws[:])
                wp_b = [P1.tile([HD, C], BF16, tag=f"wp{h}", name=f"wp{h}") for h in range(NHEADS)]
                for h in range(NHEADS):
                    ws = SCR.tile([HD, C], F32, tag="wpstage", name="wpstage")
                    nc.sync.dma_start(out=ws[:], in_=wprojTh_in[h, :, :])
                    nc.vector.tensor_copy(out=wp_b[h][:], in_=ws[:])

                qkvb_sb = P1.tile([128, 6], F32, tag="qkvb", name="qkvb")
                nc.sync.dma_start(out=qkvb_sb[:], in_=qkvb_in[:].rearrange("(o p) -> p o", p=128))
                vb_sb = P1.tile([HD, NHEADS], F32, tag="vb", name="vb")
                nc.sync.dma_start(out=vb_sb[:], in_=vb_in[:].rearrange("h p -> p h"))
                projb_sb = P1.tile([128, CT], F32, tag="projb", name="projb")
                nc.sync.dma_start(out=projb_sb[:], in_=projb_in[:].rearrange("(t p) -> p t", p=128))
                gamma_sb = P1.tile([128, CT], F32, tag="gamma", name="gamma")
                nc.sync.dma_start(out=gamma_sb[:], in_=gamma_in[:].rearrange("(t p) -> p t", p=128))
                beta_sb = P1.tile([128, CT], F32, tag="beta", name="beta")
                nc.sync.dma_start(out=beta_sb[:], in_=beta_in[:].rearrange("(t p) -> p t", p=128))
                m8_sb = [P1.tile([128, G], F32, tag=f"m8{t}", name=f"m8{t}") for t in range(CT)]
                ind8_sb = [P1.tile([G, 128], F32, tag=f"ind8{t}", name=f"ind8{t}") for t in range(CT)]
                for t in range(CT):
                    nc.sync.dma_start(out=m8_sb[t][:], in_=m8_in[t, :, :])
                    nc.sync.dma_start(out=ind8_sb[t][:], in_=ind8_in[t, :, :])

                # ---------- groupnorm ----------
                h_sb = [P1.tile([128, N], BF16, tag=f"h{t}", name=f"h{t}") for t in range(CT)]
                with (
                    tc.tile_pool(name="gn", bufs=2) as GN,
                    tc.tile_pool(name="gnps", bufs=2, space="PSUM") as GNPS,
                ):
                    FMAX = nc.vector.BN_STATS_FMAX
                    nsub = N // min(N, FMAX)
                    sub = N // nsub
                    tmp2 = []
                    for t in range(CT):
                        stats = GN.tile([128, nsub, nc.vector.BN_STATS_DIM], F32, tag="bns", name="bns")
                        for j in range(nsub):
                            nc.vector.bn_stats(
                                out=stats[:, j, :], in_=x_sb[t][:, j * sub:(j + 1) * sub]
                            )
                        mv = GN.tile([128, nc.vector.BN_AGGR_DIM], F32, tag="mv", name="mv")
                        nc.vector.bn_aggr(out=mv[:], in_=stats[:])
                        tp = GN.tile([128, 2], F32, tag=f"tmp2_{t}", name=f"tmp2_{t}")
                        nc.vector.tensor_copy(out=tp[:, 0:1], in_=mv[:, 0:1])
                        msq = GN.tile([128, 1], F32, tag="msq", name="msq")
                        nc.vector.tensor_mul(out=msq[:], in0=mv[:, 0:1], in1=mv[:, 0:1])
                        nc.vector.tensor_add(out=tp[:, 1:2], in0=mv[:, 1:2], in1=msq[:])
                        tmp2.append(tp)

                    gstat = GNPS.tile([G, 2], F32, tag="gstat", name="gstat")
                    for t in range(CT):
                        nc.tensor.matmul(
                            gstat[:], lhsT=m8_sb[t][:], rhs=tmp2[t][:],
                            start=(t == 0), stop=(t == CT - 1),
                        )
                    # gstat: col0 = mean_g, col1 = E[x^2]_g  — copy to SBUF first
                    gs = GN.tile([G, 2], F32, tag="gs", name="gs")
                    nc.vector.tensor_copy(out=gs[:], in_=gstat[:])
                    msqg = GN.tile([G, 1], F32, tag="msqg", name="msqg")
                    nc.vector.tensor_mul(out=msqg[:], in0=gs[:, 0:1], in1=gs[:, 0:1])
                    varg = GN.tile([G, 1], F32, tag="varg", name="varg")
                    nc.vector.tensor_sub(out=varg[:], in0=gs[:, 1:2], in1=msqg[:])
                    ve = GN.tile([G, 1], F32, tag="ve", name="ve")
                    nc.vector.tensor_scalar_add(out=ve[:], in0=varg[:], scalar1=EPS)
                    sq = GN.tile([G, 1], F32, tag="sq", name="sq")
                    nc.scalar.activation(out=sq[:], in_=ve[:], func=AF.Sqrt, bias=0.0, scale=1.0)
                    r0 = GN.tile([G, 1], F32, tag="r0", name="r0")
                    nc.vector.reciprocal(out=r0[:], in_=sq[:])
                    # one Newton step: r1 = r0*(1.5 - 0.5*(var+eps)*r0^2)
                    r0sq = GN.tile([G, 1], F32, tag="r0sq", name="r0sq")
                    nc.vector.tensor_mul(out=r0sq[:], in0=r0[:], in1=r0[:])
                    vr = GN.tile([G, 1], F32, tag="vr", name="vr")
                    nc.vector.tensor_mul(out=vr[:], in0=ve[:], in1=r0sq[:])
                    hh = GN.tile([G, 1], F32, tag="hh", name="hh")
                    nc.vector.tensor_scalar(
                        out=hh[:], in0=vr[:], scalar1=-0.5, scalar2=1.5,
                        op0=mybir.AluOpType.mult, op1=mybir.AluOpType.add,
                    )
                    rmr = GN.tile([G, 2], F32, tag="rmr", name="rmr")
                    nc.vector.tensor_mul(out=rmr[:, 0:1], in0=r0[:], in1=hh[:])
                    nc.vector.tensor_mul(out=rmr[:, 1:2], in0=gs[:, 0:1], in1=rmr[:, 0:1])

                    for t in range(CT):
                        bc = GNPS.tile([128, 2], F32, tag="bc", name="bc")
                        nc.tensor.matmul(bc[:], lhsT=ind8_sb[t][:], rhs=rmr[:], start=True, stop=True)
                        a_ch = GN.tile([128, 1], F32, tag=f"ach{t}", name=f"ach{t}")
                        nc.vector.tensor_mul(out=a_ch[:], in0=bc[:, 0:1], in1=gamma_sb[:, t:t + 1])
                        bg = GN.tile([128, 1], F32, tag="bg", name="bg")
                        nc.vector.tensor_mul(out=bg[:], in0=bc[:, 1:2], in1=gamma_sb[:, t:t + 1])
                        b_ch = GN.tile([128, 1], F32, tag=f"bch{t}", name=f"bch{t}")
                        nc.vector.tensor_sub(out=b_ch[:], in0=beta_sb[:, t:t + 1], in1=bg[:])
                        nc.vector.tensor_scalar(
                            out=h_sb[t][:], in0=x_sb[t][:], scalar1=a_ch[:], scalar2=b_ch[:],
                            op0=mybir.AluOpType.mult, op1=mybir.AluOpType.add,
                        )

                # ---------- qkv projections (bf16) ----------
                k_sb = [P1.tile([128, N], BF16, tag=f"k{t}", name=f"k{t}") for t in range(CT)]
                q_sb = [P1.tile([128, NHALF], BF16, tag=f"q{t}", name=f"q{t}") for t in range(CT)]
                vt_sb = [P1.tile([128, NHEADS, HD + 1], BF16, tag=f"vt{mt}", name=f"vt{mt}") for mt in range(N // 128)]
                with tc.tile_pool(name="qkvps", bufs=3, space="PSUM") as QPS:
                    def emit_q(ot):
                        for j in range(NHALF // 512):
                            ps = QPS.tile([128, 512], F32, tag="ps", name="ps")
                            for t in range(CT):
                                nc.tensor.matmul(
                                    ps[:],
                                    lhsT=wq_b[t][:, 128 * ot: 128 * ot + 128],
                                    rhs=h_sb[t][:, 512 * j: 512 * (j + 1)],
                                    start=(t == 0), stop=(t == CT - 1),
                                )
                            nc.vector.tensor_scalar_add(
                                out=q_sb[ot][:, 512 * j: 512 * (j + 1)], in0=ps[:],
                                scalar1=qkvb_sb[:, ot:ot + 1],
                            )

                    def emit_k(ot):
                        # k = W_k h (rows C..2C of qkv), full n, no bias (cancels in softmax)
                        for j in range(N // 512):
                            ps = QPS.tile([128, 512], F32, tag="ps", name="ps")
                            for t in range(CT):
                                nc.tensor.matmul(
                                    ps[:],
                                    lhsT=wq_b[t][:, C + 128 * ot: C + 128 * ot + 128],
                                    rhs=h_sb[t][:, 512 * j: 512 * (j + 1)],
                                    start=(t == 0), stop=(t == CT - 1),
                                )
                            nc.vector.tensor_copy(
                                out=k_sb[ot][:, 512 * j: 512 * (j + 1)], in_=ps[:]
                            )

                    emit_q(0)
                    emit_k(0)
                    # vT per 128-pixel tile: psum[p, h*64+d] = h^T W_v^T ; ones col appended
                    for mt in range(N // 128):
                        ps = QPS.tile([128, C], F32, tag="psv", name="psv")
                        for t in range(CT):
                            nc.tensor.matmul(
                                ps[:],
                                lhsT=h_sb[t][:, 128 * mt: 128 * (mt + 1)],
                                rhs=wq_b[t][:, 2 * C: 3 * C],
                                start=(t == 0), stop=(t == CT - 1),
                            )
                        nc.vector.tensor_copy(
                            out=vt_sb[mt][:, :, 0:HD],
                            in_=ps[:].rearrange("p (h d) -> p h d", d=HD),
                        )
                        nc.vector.memset(vt_sb[mt][:, :, HD:HD + 1], 1.0)
                    emit_q(1)
                    emit_k(1)

                # ---------- attention ----------
                att_sb = [P1.tile([HD, NHALF], BF16, tag=f"att{h}", name=f"att{h}") for h in range(NHEADS)]
                with (
                    tc.tile_pool(name="stps", bufs=2, space="PSUM") as STPS,
                    tc.tile_pool(name="avps", bufs=4, space="PSUM") as AVPS,
                    tc.tile_pool(name="pt", bufs=4) as PTP,
                    tc.tile_pool(name="rbp", bufs=2) as RBP,
                ):
                    MT = N // 128  # 32 key tiles

                    def emit_av_unit(u):
                        avs_u, hp_u, mt_u, pt_u = u[:4]
                        for hl in range(2):
                            nc.tensor.matmul(
                                avs_u[hl][0:HD + 1, :],
                                lhsT=vt_sb[mt_u][:, 2 * hp_u + hl, :],
                                rhs=pt_u[:, 512 * hl: 512 * (hl + 1)],
                                start=(mt_u == 0), stop=(mt_u == MT - 1),
                            )

                    def emit_normalize(avs_u, hp_u, nb_u):
                        for hl in range(2):
                            hg = 2 * hp_u + hl
                            av = avs_u[hl]
                            rden = RBP.tile([128, 512], F32, tag="rden", name="rden")
                            rb = RBP.tile([128, 512], F32, tag="rb", name="rb")
                            nc.vector.reciprocal(out=rden[HD:HD + 1, :], in_=av[HD:HD + 1, :])
                            # move recip row to partition 0 (DMA), then gpsimd-broadcast
                            # (partition_broadcast reads absolute partition 0 on HW)
                            nc.sync.dma_start(out=rden[0:1, :], in_=rden[HD:HD + 1, :])
                            nc.gpsimd.partition_broadcast(rb[0:HD, :], rden[0:1, :])
                            nc.vector.tensor_mul(
                                out=att_sb[hg][:, 512 * nb_u: 512 * (nb_u + 1)],
                                in0=av[0:HD, :], in1=rb[0:HD, :],
                            )
                            nc.vector.tensor_scalar_add(
                                out=att_sb[hg][:, 512 * nb_u: 512 * (nb_u + 1)],
                                in0=att_sb[hg][:, 512 * nb_u: 512 * (nb_u + 1)],
                                scalar1=vb_sb[:, hg:hg + 1],
                            )

                    # one flat software-pipelined stream over all (pass, mt) units.
                    # AV consumes pt from TWO units back: a depth-1 pipeline makes
                    # AV(u-1) wait for the in-flight exp(u-1), serializing its PE
                    # dispatch into every period; at depth 2 the PE stream never
                    # waits on the current exp.
                    DEPTH = 2
                    pend = []
                    for hp in range(2):            # head pair (2hp, 2hp+1) lives in ctile hp
                        for nb in range(NHALF // 512):
                            avs = [AVPS.tile([128, 512], F32, tag="av", name="av") for _ in range(2)]
                            for mt in range(MT):
                                st = STPS.tile([128, 1024], F32, tag="st", name="st")
                                for hl in range(2):
                                    nc.tensor.matmul(
                                        st[:, 512 * hl: 512 * (hl + 1)],
                                        lhsT=k_sb[hp][64 * hl: 64 * (hl + 1), 128 * mt: 128 * (mt + 1)],
                                        rhs=q_sb[hp][64 * hl: 64 * (hl + 1), 512 * nb: 512 * (nb + 1)],
                                        start=True, stop=True,
                                        tile_position=(64 * hl, 0),
                                    )
                                if len(pend) >= DEPTH:
                                    u = pend.pop(0)
                                    emit_av_unit(u)
                                    if u[2] == MT - 1:  # finished a pass: normalize it
                                        emit_normalize(u[0], u[1], u[4])
                                pt = PTP.tile([128, 1024], BF16, tag="pt", name="pt")
                                nc.scalar.activation(
                                    out=pt[:], in_=st[:], func=AF.Exp, scale=SCALE
                                )
                                pend.append((avs, hp, mt, pt, nb))
                    for u in pend:
                        emit_av_unit(u)
                        if u[2] == MT - 1:
                            emit_normalize(u[0], u[1], u[4])

                # ---------- proj + residual ----------
                with (
                    tc.tile_pool(name="prps", bufs=3, space="PSUM") as PRPS,
                    tc.tile_pool(name="yp", bufs=3) as YP,
                ):
                    for ot in range(CT):
                        for j in range(NHALF // 512):
                            ps = PRPS.tile([128, 512], F32, tag="ps", name="ps")
                            for h in range(NHEADS):
                                nc.tensor.matmul(
                                    ps[:],
                                    lhsT=wp_b[h][:, 128 * ot: 128 * ot + 128],
                                    rhs=att_sb[h][:, 512 * j: 512 * (j + 1)],
                                    start=(h == 0), stop=(h == NHEADS - 1),
                                )
                            y = YP.tile([128, 512], F32, tag="y", name="y")
                            nc.vector.tensor_scalar_add(
                                out=y[:], in0=ps[:], scalar1=projb_sb[:, ot:ot + 1]
                            )
                            nc.vector.tensor_add(
                                out=y[:], in0=y[:], in1=x_sb[ot][:, 512 * j: 512 * (j + 1)]
                            )
                            nc.sync.dma_start(out=y_t[ot][:, 512 * j: 512 * (j + 1)], in_=y[:])

    nc.compile()
    return nc


_CACHE = {}


def _get_nc():
    if "nc" not in _CACHE:
        _CACHE["nc"] = build_nc()
    return _CACHE["nc"]


def make_in_maps(x, gn_gamma, gn_beta, qkv_w, qkv_b, proj_w, proj_b):
    x = np.asarray(x, dtype=np.float32)
    qkv_w = np.asarray(qkv_w, dtype=np.float32)
    qkv_b = np.asarray(qkv_b, dtype=np.float32)
    proj_w = np.asarray(proj_w, dtype=np.float32)
    proj_b = np.asarray(proj_b, dtype=np.float32)
    gn_gamma = np.asarray(gn_gamma, dtype=np.float32)
    gn_beta = np.asarray(gn_beta, dtype=np.float32)

    wqkvT = np.ascontiguousarray(qkv_w.T)                       # [C, 3C]
    wprojTh = np.ascontiguousarray(proj_w.T.reshape(NHEADS, HD, C))  # [4, 64, C]
    vb = np.ascontiguousarray(qkv_b[2 * C:].reshape(NHEADS, HD))

    cidx = np.arange(128)
    m8 = np.zeros((CT, 128, G), np.float32)
    ind8 = np.zeros((CT, G, 128), np.float32)
    for t in range(CT):
        g = 4 * t + cidx // 32
        m8[t, cidx, g] = 1.0 / (C // G)  # tmp2 holds per-row means: average 32 rows per group
        ind8[t, g, cidx] = 1.0

    in_maps = []
    for core in range(8):
        b, half = core // 2, core % 2
        xb = x[b].reshape(C, N)
        if half == 1:
            # rotate so this core's pixel-half sits in columns 0:NHALF.
            # GroupNorm stats and softmax-over-keys are pixel-permutation
            # invariant, so k/v built from the rotated image are equivalent.
            xb = np.concatenate([xb[:, NHALF:], xb[:, :NHALF]], axis=1)
        in_maps.append({
            "xb": np.ascontiguousarray(xb), "wqkvT": wqkvT, "wprojTh": wprojTh,
            "qkvb": qkv_b, "vb": vb, "projb": proj_b,
            "gamma": gn_gamma, "beta": gn_beta, "m8": m8, "ind8": ind8,
        })
    return in_maps


def assemble(results):
    y = np.empty((B, C, N), np.float32)
    for core in range(8):
        b, half = core // 2, core % 2
        y[b][:, half * NHALF:(half + 1) * NHALF] = results[core]["y"]
    return y.reshape(B, C, H, W)


def kernel(x, gn_gamma, gn_beta, qkv_w, qkv_b, proj_w, proj_b):
    nc = _get_nc()
    in_maps = make_in_maps(x, gn_gamma, gn_beta, qkv_w, qkv_b, proj_w, proj_b)
    last_err = None
    for attempt in range(3):
        try:
            res = run_bass_kernel_spmd(nc, in_maps, list(range(8)))
            return assemble(res.results)
        except Exception as e:  # transient NRT_EXEC_UNIT_UNRECOVERABLE hiccups
            last_err = e
            import time as _time
            _time.sleep(5)
    raise last_err



# revision 20
# speedup vs baseline: 3873.3300x; 3873.3300x over previous
"""Trainium2 Bass kernel for nn_AttentionBlock (GroupNorm + 4-head self-attention + proj).

Sharding: 8 cores; core i handles batch b=i//2 and pixel-half i%2 (2048 of 4096
pixels). Each core computes GroupNorm + k,v over the full batch image (needed
for attention over all keys), q/attention/proj only for its pixel half.
Host-side work is limited to slicing, transposing weight layouts, and
concatenating the 8 output shards.
"""

import sys

sys.path.insert(0, "/opt/trn_rl_repo")

import numpy as np

import concourse.bass as bass
import concourse.mybir as mybir
import concourse.tile as tile
from concourse import bacc
from concourse.bass_utils import run_bass_kernel_spmd

F32 = mybir.dt.float32
BF16 = mybir.dt.bfloat16
AF = mybir.ActivationFunctionType

B, C, H, W = 4, 256, 64, 64
N = H * W          # 4096 pixels
NHALF = N // 2     # 2048 per core
G = 8              # groupnorm groups
NHEADS = 4
HD = C // NHEADS   # 64
CT = C // 128      # 2 channel tiles of 128
SCALE = HD ** -0.5
EPS = 1e-5
CNT = (C // G) * N  # elements per group (per batch)


def build_nc(reps=1):
    nc = bacc.Bacc(None, target_bir_lowering=False)

    x_in = nc.declare_dram_parameter("xb", [C, N], F32, isOutput=False)
    wqkvT_in = nc.declare_dram_parameter("wqkvT", [C, 3 * C], F32, isOutput=False)
    wprojTh_in = nc.declare_dram_parameter("wprojTh", [NHEADS, HD, C], F32, isOutput=False)
    qkvb_in = nc.declare_dram_parameter("qkvb", [3 * C], F32, isOutput=False)
    vb_in = nc.declare_dram_parameter("vb", [NHEADS, HD], F32, isOutput=False)
    projb_in = nc.declare_dram_parameter("projb", [C], F32, isOutput=False)
    gamma_in = nc.declare_dram_parameter("gamma", [C], F32, isOutput=False)
    beta_in = nc.declare_dram_parameter("beta", [C], F32, isOutput=False)
    m8_in = nc.declare_dram_parameter("m8", [CT, 128, G], F32, isOutput=False)
    ind8_in = nc.declare_dram_parameter("ind8", [CT, G, 128], F32, isOutput=False)
    y_out = nc.declare_dram_parameter("y", [C, NHALF], F32, isOutput=True)

    x_t = x_in[:].rearrange("(t p) n -> t p n", p=128)
    w_t = wqkvT_in[:].rearrange("(t p) o -> t p o", p=128)
    y_t = y_out[:].rearrange("(t p) n -> t p n", p=128)

    with tile.TileContext(nc) as tc:
        with (
            tc.tile_pool(name="persist", bufs=1) as P1,
            tc.tile_pool(name="scratch", bufs=2) as SCR,
        ):
            import contextlib
            loop_cm = tc.For_i(0, reps, 1) if reps > 1 else contextlib.nullcontext()
            with loop_cm:
                # ---------- load ----------
                x_sb = [P1.tile([128, N], F32, tag=f"x{t}", name=f"x{t}") for t in range(CT)]
                NCH = 4  # chunked load so groupnorm stats can start early
                for t in range(CT):
                    for jc in range(NCH):
                        nc.sync.dma_start(
                            out=x_sb[t][:, jc * (N // NCH):(jc + 1) * (N // NCH)],
                            in_=x_t[t][:, jc * (N // NCH):(jc + 1) * (N // NCH)],
                        )

                wq_b = [P1.tile([128, 3 * C], BF16, tag=f"wq{t}", name=f"wq{t}") for t in range(CT)]
                for t in range(CT):
                    ws = SCR.tile([128, 3 * C], F32, tag="wstage", name="wstage")
                    nc.sync.dma_start(out=ws[:], in_=w_t[t])
                    nc.vector.tensor_copy(out=wq_b[t][:], in_=ws[:])
                wp_b = [P1.tile([HD, C], BF16, tag=f"wp{h}", name=f"wp{h}") for h in range(NHEADS)]
                for h in range(NHEADS):
                    ws = SCR.tile([HD, C], F32, tag="wpstage", name="wpstage")
                    nc.sync.dma_start(out=ws[:], in_=wprojTh_in[h, :, :])
                    nc.vector.tensor_copy(out=wp_b[h][:], in_=ws[:])

                qkvb_sb = P1.tile([128, 6], F32, tag="qkvb", name="qkvb")
                nc.sync.dma_start(out=qkvb_sb[:], in_=qkvb_in[:].rearrange("(o p) -> p o", p=128))
                vb_sb = P1.tile([HD, NHEADS], F32, tag="vb", name="vb")
                nc.sync.dma_start(out=vb_sb[:], in_=vb_in[:].rearrange("h p -> p h"))
                projb_sb = P1.tile([128, CT], F32, tag="projb", name="projb")
                nc.sync.dma_start(out=projb_sb[:], in_=projb_in[:].rearrange("(t p) -> p t", p=128))
                gamma_sb = P1.tile([128, CT], F32, tag="gamma", name="gamma")
                nc.sync.dma_start(out=gamma_sb[:], in_=gamma_in[:].rearrange("(t p) -> p t", p=128))
                beta_sb = P1.tile([128, CT], F32, tag="beta", name="beta")
                nc.sync.dma_start(out=beta_sb[:], in_=beta_in[:].rearrange("(t p) -> p t", p=128))
                m8_sb = [P1.tile([128, G], F32, tag=f"m8{t}", name=f"m8{t}") for t in range(CT)]
                ind8_sb = [P1.tile([G, 128], F32, tag=f"ind8{t}", name=f"ind8{t}") for t in range(CT)]
                for t in range(CT):
                    nc.sync.dma_start(out=m8_sb[t][:], in_=m8_in[t, :, :])
                    nc.sync.dma_start(out=ind8_sb[t][:], in_=ind8_in[t, :, :])

                # ---------- groupnorm ----------
                h_sb = [P1.tile([128, N], BF16, tag=f"h{t}", name=f"h{t}") for t in range(CT)]
                with (
                    tc.tile_pool(name="gn", bufs=2) as GN,
                    tc.tile_pool(name="gnps", bufs=2, space="PSUM") as GNPS,
                ):
                    FMAX = nc.vector.BN_STATS_FMAX
                    nsub = N // min(N, FMAX)
                    sub = N // nsub
                    tmp2 = []
                    for t in range(CT):
                        stats = GN.tile([128, nsub, nc.vector.BN_STATS_DIM], F32, tag="bns", name="bns")
                        for j in range(nsub):
                            nc.vector.bn_stats(
                                out=stats[:, j, :], in_=x_sb[t][:, j * sub:(j + 1) * sub]
                            )
                        mv = GN.tile([128, nc.vector.BN_AGGR_DIM], F32, tag="mv", name="mv")
                        nc.vector.bn_aggr(out=mv[:], in_=stats[:])
                        tp = GN.tile([128, 2], F32, tag=f"tmp2_{t}", name=f"tmp2_{t}")
                        nc.vector.tensor_copy(out=tp[:, 0:1], in_=mv[:, 0:1])
                        msq = GN.tile([128, 1], F32, tag="msq", name="msq")
                        nc.vector.tensor_mul(out=msq[:], in0=mv[:, 0:1], in1=mv[:, 0:1])
                        nc.vector.tensor_add(out=tp[:, 1:2], in0=mv[:, 1:2], in1=msq[:])
                        tmp2.append(tp)

                    gstat = GNPS.tile([G, 2], F32, tag="gstat", name="gstat")
                    for t in range(CT):
                        nc.tensor.matmul(
                            gstat[:], lhsT=m8_sb[t][:], rhs=tmp2[t][:],
                            start=(t == 0), stop=(t == CT - 1),
                        )
                    # gstat: col0 = mean_g, col1 = E[x^2]_g  — copy to SBUF first
                    gs = GN.tile([G, 2], F32, tag="gs", name="gs")
                    nc.vector.tensor_copy(out=gs[:], in_=gstat[:])
                    msqg = GN.tile([G, 1], F32, tag="msqg", name="msqg")
                    nc.vector.tensor_mul(out=msqg[:], in0=gs[:, 0:1], in1=gs[:, 0:1])
                    varg = GN.tile([G, 1], F32, tag="varg", name="varg")
                    nc.vector.tensor_sub(out=varg[:], in0=gs[:, 1:2], in1=msqg[:])
                    ve = GN.tile([G, 1], F32, tag="ve", name="ve")
                    nc.vector.tensor_scalar_add(out=ve[:], in0=varg[:], scalar1=EPS)
                    sq = GN.tile([G, 1], F32, tag="sq", name="sq")
                    nc.scalar.activation(out=sq[:], in_=ve[:], func=AF.Sqrt, bias=0.0, scale=1.0)
                    r0 = GN.tile([G, 1], F32, tag="r0", name="r0")
                    nc.vector.reciprocal(out=r0[:], in_=sq[:])
                    # one Newton step: r1 = r0*(1.5 - 0.5*(var+eps)*r0^2)
                    r0sq = GN.tile([G, 1], F32, tag="r0sq", name="r0sq")
                    nc.vector.tensor_mul(out=r0sq[:], in0=r0[:], in1=r0[:])
                    vr = GN.tile([G, 1], F32, tag="vr", name="vr")
                    nc.vector.tensor_mul(out=vr[:], in0=ve[:], in1=r0sq[:])
                    hh = GN.tile([G, 1], F32, tag="hh", name="hh")
                    nc.vector.tensor_scalar(
                        out=hh[:], in0=vr[:], scalar1=-0.5, scalar2=1.5,
                        op0=mybir.AluOpType.mult, op1=mybir.AluOpType.add,
                    )
                    rmr = GN.tile([G, 2], F32, tag="rmr", name="rmr")
                    nc.vector.tensor_mul(out=rmr[:, 0:1], in0=r0[:], in1=hh[:])
                    nc.vector.tensor_mul(out=rmr[:, 1:2], in0=gs[:, 0:1], in1=rmr[:, 0:1])

                    for t in range(CT):
                        bc = GNPS.tile([128, 2], F32, tag="bc", name="bc")
                        nc.tensor.matmul(bc[:], lhsT=ind8_sb[t][:], rhs=rmr[:], start=True, stop=True)
                        a_ch = GN.tile([128, 1], F32, tag=f"ach{t}", name=f"ach{t}")
                        nc.vector.tensor_mul(out=a_ch[:], in0=bc[:, 0:1], in1=gamma_sb[:, t:t + 1])
                        bg = GN.tile([128, 1], F32, tag="bg", name="bg")
                        nc.vector.tensor_mul(out=bg[:], in0=bc[:, 1:2], in1=gamma_sb[:, t:t + 1])
                        b_ch = GN.tile([128, 1], F32, tag=f"bch{t}", name=f"bch{t}")
                        nc.vector.tensor_sub(out=b_ch[:], in0=beta_sb[:, t:t + 1], in1=bg[:])
                        for jc in range(N // 512):
                            nc.vector.tensor_scalar(
                                out=h_sb[t][:, 512 * jc: 512 * (jc + 1)],
                                in0=x_sb[t][:, 512 * jc: 512 * (jc + 1)],
                                scalar1=a_ch[:], scalar2=b_ch[:],
                                op0=mybir.AluOpType.mult, op1=mybir.AluOpType.add,
                            )

                # ---------- qkv projections (bf16) ----------
                k_sb = [P1.tile([128, N], BF16, tag=f"k{t}", name=f"k{t}") for t in range(CT)]
                q_sb = [P1.tile([128, NHALF], BF16, tag=f"q{t}", name=f"q{t}") for t in range(CT)]
                vt_sb = [P1.tile([128, NHEADS, HD + 1], BF16, tag=f"vt{mt}", name=f"vt{mt}") for mt in range(N // 128)]
                with tc.tile_pool(name="qkvps", bufs=3, space="PSUM") as QPS:
                    def emit_q(ot):
                        for j in range(NHALF // 512):
                            ps = QPS.tile([128, 512], F32, tag="ps", name="ps")
                            for t in range(CT):
                                nc.tensor.matmul(
                                    ps[:],
                                    lhsT=wq_b[t][:, 128 * ot: 128 * ot + 128],
                                    rhs=h_sb[t][:, 512 * j: 512 * (j + 1)],
                                    start=(t == 0), stop=(t == CT - 1),
                                )
                            nc.vector.tensor_scalar_add(
                                out=q_sb[ot][:, 512 * j: 512 * (j + 1)], in0=ps[:],
                                scalar1=qkvb_sb[:, ot:ot + 1],
                            )

                    def emit_k(ot):
                        # k = W_k h (rows C..2C of qkv), full n, no bias (cancels in softmax)
                        for j in range(N // 512):
                            ps = QPS.tile([128, 512], F32, tag="ps", name="ps")
                            for t in range(CT):
                                nc.tensor.matmul(
                                    ps[:],
                                    lhsT=wq_b[t][:, C + 128 * ot: C + 128 * ot + 128],
                                    rhs=h_sb[t][:, 512 * j: 512 * (j + 1)],
                                    start=(t == 0), stop=(t == CT - 1),
                                )
                            nc.vector.tensor_copy(
                                out=k_sb[ot][:, 512 * j: 512 * (j + 1)], in_=ps[:]
                            )

                    emit_q(0)
                    emit_k(0)
                    # vT per 128-pixel tile: psum[p, h*64+d] = h^T W_v^T ; ones col appended
                    for mt in range(N // 128):
                        ps = QPS.tile([128, C], F32, tag="psv", name="psv")
                        for t in range(CT):
                            nc.tensor.matmul(
                                ps[:],
                                lhsT=h_sb[t][:, 128 * mt: 128 * (mt + 1)],
                                rhs=wq_b[t][:, 2 * C: 3 * C],
                                start=(t == 0), stop=(t == CT - 1),
                            )
                        nc.vector.tensor_copy(
                            out=vt_sb[mt][:, :, 0:HD],
                            in_=ps[:].rearrange("p (h d) -> p h d", d=HD),
                        )
                        nc.vector.memset(vt_sb[mt][:, :, HD:HD + 1], 1.0)
                    emit_q(1)
                    emit_k(1)

                # ---------- attention ----------
                att_sb = [P1.tile([HD, NHALF], BF16, tag=f"att{h}", name=f"att{h}") for h in range(NHEADS)]
                with (
                    tc.tile_pool(name="stps", bufs=2, space="PSUM") as STPS,
                    tc.tile_pool(name="avps", bufs=4, space="PSUM") as AVPS,
                    tc.tile_pool(name="pt", bufs=4) as PTP,
                    tc.tile_pool(name="rbp", bufs=2) as RBP,
                    tc.tile_pool(name="yp", bufs=3) as YP,
                ):
                    def emit_proj(nb_u):
                        # att[:, nb] complete for all heads; psum from the AV pool
                        for ot in range(CT):
                            ps = AVPS.tile([128, 512], F32, tag="av", name="av")
                            for h in range(NHEADS):
                                nc.tensor.matmul(
                                    ps[:],
                                    lhsT=wp_b[h][:, 128 * ot: 128 * ot + 128],
                                    rhs=att_sb[h][:, 512 * nb_u: 512 * (nb_u + 1)],
                                    start=(h == 0), stop=(h == NHEADS - 1),
                                )
                            y = YP.tile([128, 512], F32, tag="y", name="y")
                            nc.vector.tensor_scalar_add(
                                out=y[:], in0=ps[:], scalar1=projb_sb[:, ot:ot + 1]
                            )
                            nc.vector.tensor_add(
                                out=y[:], in0=y[:], in1=x_sb[ot][:, 512 * nb_u: 512 * (nb_u + 1)]
                            )
                            nc.sync.dma_start(out=y_t[ot][:, 512 * nb_u: 512 * (nb_u + 1)], in_=y[:])

                    MT = N // 128  # 32 key tiles

                    def emit_av_unit(u):
                        avs_u, hp_u, mt_u, pt_u = u[:4]
                        for hl in range(2):
                            nc.tensor.matmul(
                                avs_u[hl][0:HD + 1, :],
                                lhsT=vt_sb[mt_u][:, 2 * hp_u + hl, :],
                                rhs=pt_u[:, 512 * hl: 512 * (hl + 1)],
                                start=(mt_u == 0), stop=(mt_u == MT - 1),
                            )

                    def emit_normalize(avs_u, hp_u, nb_u):
                        for hl in range(2):
                            hg = 2 * hp_u + hl
                            av = avs_u[hl]
                            rden = RBP.tile([128, 512], F32, tag="rden", name="rden")
                            rb = RBP.tile([128, 512], F32, tag="rb", name="rb")
                            nc.vector.reciprocal(out=rden[HD:HD + 1, :], in_=av[HD:HD + 1, :])
                            # move recip row to partition 0 (DMA), then gpsimd-broadcast
                            # (partition_broadcast reads absolute partition 0 on HW)
                            nc.sync.dma_start(out=rden[0:1, :], in_=rden[HD:HD + 1, :])
                            nc.gpsimd.partition_broadcast(rb[0:HD, :], rden[0:1, :])
                            nc.vector.tensor_mul(
                                out=att_sb[hg][:, 512 * nb_u: 512 * (nb_u + 1)],
                                in0=av[0:HD, :], in1=rb[0:HD, :],
                            )
                            nc.vector.tensor_scalar_add(
                                out=att_sb[hg][:, 512 * nb_u: 512 * (nb_u + 1)],
                                in0=att_sb[hg][:, 512 * nb_u: 512 * (nb_u + 1)],
                                scalar1=vb_sb[:, hg:hg + 1],
                            )

                    # one flat software-pipelined stream over all (pass, mt) units.
                    # AV consumes pt from TWO units back: a depth-1 pipeline makes
                    # AV(u-1) wait for the in-flight exp(u-1), serializing its PE
                    # dispatch into every period; at depth 2 the PE stream never
                    # waits on the current exp.
                    DEPTH = 2
                    pend = []
                    for hp in range(2):            # head pair (2hp, 2hp+1) lives in ctile hp
                        for nb in range(NHALF // 512):
                            avs = [AVPS.tile([128, 512], F32, tag="av", name="av") for _ in range(2)]
                            for mt in range(MT):
                                st = STPS.tile([128, 1024], F32, tag="st", name="st")
                                for hl in range(2):
                                    nc.tensor.matmul(
                                        st[:, 512 * hl: 512 * (hl + 1)],
                                        lhsT=k_sb[hp][64 * hl: 64 * (hl + 1), 128 * mt: 128 * (mt + 1)],
                                        rhs=q_sb[hp][64 * hl: 64 * (hl + 1), 512 * nb: 512 * (nb + 1)],
                                        start=True, stop=True,
                                        tile_position=(64 * hl, 0),
                                    )
                                if len(pend) >= DEPTH:
                                    u = pend.pop(0)
                                    emit_av_unit(u)
                                    if u[2] == MT - 1:  # finished a pass: normalize it
                                        emit_normalize(u[0], u[1], u[4])
                                        if u[1] == 1:   # both head-pairs done for this nb
                                            emit_proj(u[4])
                                pt = PTP.tile([128, 1024], BF16, tag="pt", name="pt")
                                nc.scalar.activation(
                                    out=pt[:], in_=st[:], func=AF.Exp, scale=SCALE
                                )
                                pend.append((avs, hp, mt, pt, nb))
                    for u in pend:
                        emit_av_unit(u)
                        if u[2] == MT - 1:
                            emit_normalize(u[0], u[1], u[4])
                            if u[1] == 1:
                                emit_proj(u[4])

    nc.compile()
    return nc


_CACHE = {}


def _get_nc():
    if "nc" not in _CACHE:
        _CACHE["nc"] = build_nc()
    return _CACHE["nc"]


def make_in_maps(x, gn_gamma, gn_beta, qkv_w, qkv_b, proj_w, proj_b):
    x = np.asarray(x, dtype=np.float32)
    qkv_w = np.asarray(qkv_w, dtype=np.float32)
    qkv_b = np.asarray(qkv_b, dtype=np.float32)
    proj_w = np.asarray(proj_w, dtype=np.float32)
    proj_b = np.asarray(proj_b, dtype=np.float32)
    gn_gamma = np.asarray(gn_gamma, dtype=np.float32)
    gn_beta = np.asarray(gn_beta, dtype=np.float32)

    wqkvT = np.ascontiguousarray(qkv_w.T)                       # [C, 3C]
    wprojTh = np.ascontiguousarray(proj_w.T.reshape(NHEADS, HD, C))  # [4, 64, C]
    vb = np.ascontiguousarray(qkv_b[2 * C:].reshape(NHEADS, HD))

    cidx = np.arange(128)
    m8 = np.zeros((CT, 128, G), np.float32)
    ind8 = np.zeros((CT, G, 128), np.float32)
    for t in range(CT):
        g = 4 * t + cidx // 32
        m8[t, cidx, g] = 1.0 / (C // G)  # tmp2 holds per-row means: average 32 rows per group
        ind8[t, g, cidx] = 1.0

    in_maps = []
    for core in range(8):
        b, half = core // 2, core % 2
        xb = x[b].reshape(C, N)
        if half == 1:
            # rotate so this core's pixel-half sits in columns 0:NHALF.
            # GroupNorm stats and softmax-over-keys are pixel-permutation
            # invariant, so k/v built from the rotated image are equivalent.
            xb = np.concatenate([xb[:, NHALF:], xb[:, :NHALF]], axis=1)
        in_maps.append({
            "xb": np.ascontiguousarray(xb), "wqkvT": wqkvT, "wprojTh": wprojTh,
            "qkvb": qkv_b, "vb": vb, "projb": proj_b,
            "gamma": gn_gamma, "beta": gn_beta, "m8": m8, "ind8": ind8,
        })
    return in_maps


def assemble(results):
    y = np.empty((B, C, N), np.float32)
    for core in range(8):
        b, half = core // 2, core % 2
        y[b][:, half * NHALF:(half + 1) * NHALF] = results[core]["y"]
    return y.reshape(B, C, H, W)


def kernel(x, gn_gamma, gn_beta, qkv_w, qkv_b, proj_w, proj_b):
    nc = _get_nc()
    in_maps = make_in_maps(x, gn_gamma, gn_beta, qkv_w, qkv_b, proj_w, proj_b)
    last_err = None
    for attempt in range(3):
        try:
            res = run_bass_kernel_spmd(nc, in_maps, list(range(8)))
            return assemble(res.results)
        except Exception as e:  # transient NRT_EXEC_UNIT_UNRECOVERABLE hiccups
            last_err = e
            import time as _time
            _time.sleep(5)
    raise last_err

